# revision 7
# baseline (speedup 1.0000x reference)
"""Trainium2 Bass kernel for nn_ContrastiveLoss3DTo2D.

Reference computation (B=256, D=1024, margin=0.2):
    scores[i, j] = dot(im[j], s[i, j])                    # [B, B]
    cost_s  = sum_i relu(margin + max_{j!=i} scores[i,j] - scores[i,i])
    cost_im = sum_j relu(margin + max_{i!=j} scores[i,j] - scores[j,j])
    loss = cost_s + cost_im

Sharding: s (and the score matrix) is sharded along i across 8 cores
(32 rows each); im is replicated. Each core computes its 32x256 score
block via a fused DVE multiply+reduce while streaming its 32 MB shard
from HBM, then reduces on-device to tiny per-core partials:
  rowcost[32]  - relu(margin + rowmax_offdiag - diag) per local row
  colmax[256]  - per-column max over local rows (diagonal masked out)
  diagv[32]    - local diagonal scores
The host combines partials (max over cores for columns, sums) into the
scalar loss. relu/max commute (relu is monotone), so per-core column
maxima compose exactly.
"""

import numpy as np

B = 256
D = 1024
M = 8            # cores
BL = B // M      # 32 local rows per core
P = 128          # SBUF partitions
T = B // P       # 2 column tiles of 128
MARGIN = 0.2
CHUNK = 4        # s rows per DMA (4 MB transfers)
NEG = -1.0e30    # diagonal mask value
NEG_INIT = -3.0e38

_NC = None


def _build_nc():
    import concourse.bacc as bacc
    from concourse import mybir
    from concourse.tile import TileContext

    f32 = mybir.dt.float32
    add = mybir.AluOpType.add
    mult = mybir.AluOpType.mult
    amax = mybir.AluOpType.max

    nc = bacc.Bacc(None, target_bir_lowering=False, debug=False)
    im_d = nc.declare_dram_parameter("im", [B, D], f32, isOutput=False)
    s_d = nc.declare_dram_parameter("s", [BL, B, D], f32, isOutput=False)
    mt_d = nc.declare_dram_parameter("mask_t_neg", [B, BL], f32, isOutput=False)
    nr_d = nc.declare_dram_parameter("neg_rows", [BL, B], f32, isOutput=False)
    er_d = nc.declare_dram_parameter("eye_rows", [BL, B], f32, isOutput=False)
    rc_d = nc.declare_dram_parameter("rowcost", [BL, 1], f32, isOutput=True)
    cm_d = nc.declare_dram_parameter("colmax", [B, 1], f32, isOutput=True)
    dg_d = nc.declare_dram_parameter("diagv", [BL, 1], f32, isOutput=True)

    with TileContext(nc) as tc:
        with (
            tc.tile_pool(name="const", bufs=1) as cpool,
            tc.tile_pool(name="sload", bufs=3) as spool,
            tc.tile_pool(name="scratch", bufs=4) as prpool,
            tc.tile_pool(name="small", bufs=1) as smpool,
        ):
            # im packed as [p, t*D + d] so j = t*128 + p matches the s tiles.
            im_t = cpool.tile([P, T * D], f32, tag="im")
            nc.sync.dma_start(
                out=im_t[:].rearrange("p (t d) -> p t d", t=T),
                in_=im_d[:].rearrange("(t p) d -> p t d", p=P),
            )
            mt_t = cpool.tile([P, T * BL], f32, tag="maskT")
            nc.sync.dma_start(
                out=mt_t[:].rearrange("p (t i) -> p t i", t=T),
                in_=mt_d[:].rearrange("(t p) i -> p t i", p=P),
            )
            nr_t = cpool.tile([BL, B], f32, tag="negrows")
            nc.sync.dma_start(out=nr_t[:], in_=nr_d[:])
            er_t = cpool.tile([BL, B], f32, tag="eyerows")
            nc.sync.dma_start(out=er_t[:], in_=er_d[:])

            # scores^T: partition = column j (within tile t), free = local row i
            scoresT = smpool.tile([P, T * BL], f32, tag="scoresT")

            for ch in range(BL // CHUNK):
                s_t = spool.tile([P, CHUNK * T * D], f32, tag="s")
                nc.sync.dma_start(
                    out=s_t[:].rearrange("p (r t d) -> p r t d", r=CHUNK, t=T),
                    in_=s_d[ch * CHUNK:(ch + 1) * CHUNK].rearrange(
                        "r (t p) d -> p r t d", p=P
                    ),
                )
                for r in range(CHUNK):
                    i = ch * CHUNK + r
                    off = r * T * D
                    prod = prpool.tile([P, T * D], f32, tag="prod")
                    # DVE: one fused product over both column halves.
                    nc.vector.tensor_mul(
                        prod[:], s_t[:, off:off + T * D], im_t[:]
                    )
                    # Free-axis sums: ACT accumulates most halves; DVE
                    # takes every 4th i's t=1 half to balance engine load.
                    nc.scalar.activation(
                        out=prod[:, 0:D], in_=prod[:, 0:D],
                        func=mybir.ActivationFunctionType.Copy,
                        accum_out=scoresT[:, i:i + 1],
                    )
                    if i % 4 == 3:
                        nc.vector.reduce_sum(
                            scoresT[:, BL + i:BL + i + 1], prod[:, D:2 * D],
                            axis=mybir.AxisListType.X,
                        )
                    else:
                        nc.scalar.activation(
                            out=prod[:, D:2 * D], in_=prod[:, D:2 * D],
                            func=mybir.ActivationFunctionType.Copy,
                            accum_out=scoresT[:, BL + i:BL + i + 1],
                        )

            # Column maxima over local rows, diagonal masked to -1e30.
            colmax = smpool.tile([P, T], f32, tag="colmax")
            for t in range(T):
                mscr = prpool.tile([P, BL], f32, tag="mscr")
                nc.vector.tensor_add(
                    mscr[:],
                    scoresT[:, t * BL:(t + 1) * BL],
                    mt_t[:, t * BL:(t + 1) * BL],
                )
                nc.vector.reduce_max(
                    colmax[:, t:t + 1], mscr[:], axis=mybir.AxisListType.X
                )

            # Transpose scores^T -> rows [32, 256] via 32x32 stream blocks.
            rows = smpool.tile([BL, B], f32, tag="rows")
            for t in range(T):
                for k in range(P // 32):
                    nc.vector.transpose(
                        out=rows[0:BL, t * P + k * 32:t * P + (k + 1) * 32],
                        in_=scoresT[k * 32:(k + 1) * 32, t * BL:(t + 1) * BL],
                    )

            # rowmax (diag masked), diag, then rowcost = relu(margin + rowmax - diag)
            rowstat = smpool.tile([BL, 4], f32, tag="rowstat")
            rs1 = prpool.tile([BL, B], f32, tag="rscr")
            nc.vector.tensor_add(rs1[:], rows[:], nr_t[:])
            nc.vector.reduce_max(
                rowstat[:, 0:1], rs1[:], axis=mybir.AxisListType.X
            )
            rs2 = prpool.tile([BL, B], f32, tag="rscr")
            nc.vector.tensor_mul(rs2[:], rows[:], er_t[:])
            nc.vector.reduce_sum(
                rowstat[:, 1:2], rs2[:], axis=mybir.AxisListType.X
            )
            nc.vector.tensor_sub(rowstat[:, 2:3], rowstat[:, 0:1], rowstat[:, 1:2])
            nc.vector.tensor_scalar(
                out=rowstat[:, 3:4], in0=rowstat[:, 2:3],
                scalar1=MARGIN, scalar2=0.0, op0=add, op1=amax,
            )

            nc.sync.dma_start(out=rc_d[:], in_=rowstat[:, 3:4])
            nc.sync.dma_start(out=dg_d[:], in_=rowstat[:, 1:2])
            for t in range(T):
                nc.sync.dma_start(
                    out=cm_d[t * P:(t + 1) * P], in_=colmax[:, t:t + 1]
                )

    nc.compile()
    return nc


def _get_nc():
    global _NC
    if _NC is None:
        _NC = _build_nc()
    return _NC


def _make_in_maps(im, s):
    il = np.arange(BL)
    in_maps = []
    for c in range(M):
        mt = np.zeros((B, BL), np.float32)
        nr = np.zeros((BL, B), np.float32)
        er = np.zeros((BL, B), np.float32)
        mt[c * BL + il, il] = NEG
        nr[il, c * BL + il] = NEG
        er[il, c * BL + il] = 1.0
        in_maps.append({
            "im": im,
            "s": s[c * BL:(c + 1) * BL],
            "mask_t_neg": mt,
            "neg_rows": nr,
            "eye_rows": er,
        })
    return in_maps


def _combine(results):
    rowcosts = np.concatenate([results[c]["rowcost"][:, 0] for c in range(M)])
    colmax = np.max(
        np.stack([results[c]["colmax"][:, 0] for c in range(M)]), axis=0
    )
    diag = np.concatenate([results[c]["diagv"][:, 0] for c in range(M)])
    cost_im = np.maximum(np.float32(MARGIN) + colmax - diag, np.float32(0.0))
    loss = rowcosts.sum(dtype=np.float32) + cost_im.sum(dtype=np.float32)
    return np.array(loss, dtype=np.float32)


def _run(im, s, **spmd_kwargs):
    from concourse.bass_utils import run_bass_kernel_spmd

    im = np.ascontiguousarray(np.asarray(im), dtype=np.float32)
    s = np.ascontiguousarray(np.asarray(s), dtype=np.float32)
    nc = _get_nc()
    res = run_bass_kernel_spmd(nc, _make_in_maps(im, s), list(range(M)),
                               **spmd_kwargs)
    return _combine(res.results), res


def kernel(im, s):
    loss, _ = _run(im, s)
    return loss


# revision 11
# speedup vs baseline: 1.0729x; 1.0729x over previous
"""Trainium2 Bass kernel for nn_ContrastiveLoss3DTo2D.

Reference computation (B=256, D=1024, margin=0.2):
    scores[i, j] = dot(im[j], s[i, j])                    # [B, B]
    cost_s  = sum_i relu(margin + max_{j!=i} scores[i,j] - scores[i,i])
    cost_im = sum_j relu(margin + max_{i!=j} scores[i,j] - scores[j,j])
    loss = cost_s + cost_im

Sharding: s (and the score matrix) is sharded along i across 8 cores
(32 rows each); im is replicated. Each core computes its 32x256 score
block via a fused DVE multiply+reduce while streaming its 32 MB shard
from HBM, then reduces on-device to tiny per-core partials:
  rowcost[32]  - relu(margin + rowmax_offdiag - diag) per local row
  colmax[256]  - per-column max over local rows (diagonal masked out)
  diagv[32]    - local diagonal scores
The host combines partials (max over cores for columns, sums) into the
scalar loss. relu/max commute (relu is monotone), so per-core column
maxima compose exactly.
"""

import numpy as np

B = 256
D = 1024
M = 8            # cores
BL = B // M      # 32 local rows per core
P = 128          # SBUF partitions
T = B // P       # 2 column tiles of 128
MARGIN = 0.2
CHUNK = 4        # s rows per DMA (4 MB transfers)
NEG = -1.0e30    # diagonal mask value
NEG_INIT = -3.0e38

_NC = None


def _build_nc():
    import concourse.bacc as bacc
    from concourse import mybir
    from concourse.tile import TileContext

    f32 = mybir.dt.float32
    add = mybir.AluOpType.add
    mult = mybir.AluOpType.mult
    amax = mybir.AluOpType.max

    nc = bacc.Bacc(None, target_bir_lowering=False, debug=False)
    im_d = nc.declare_dram_parameter("im", [B, D], f32, isOutput=False)
    s_d = nc.declare_dram_parameter("s", [BL, B, D], f32, isOutput=False)
    mt_d = nc.declare_dram_parameter("mask_t_neg", [B, BL], f32, isOutput=False)
    nr_d = nc.declare_dram_parameter("neg_rows", [BL, B], f32, isOutput=False)
    er_d = nc.declare_dram_parameter("eye_rows", [BL, B], f32, isOutput=False)
    rc_d = nc.declare_dram_parameter("rowcost", [BL, 1], f32, isOutput=True)
    cm_d = nc.declare_dram_parameter("colmax", [B, 1], f32, isOutput=True)
    dg_d = nc.declare_dram_parameter("diagv", [BL, 1], f32, isOutput=True)

    with TileContext(nc) as tc:
        with (
            tc.tile_pool(name="const", bufs=1) as cpool,
            tc.tile_pool(name="sload", bufs=4) as spool,
            tc.tile_pool(name="scratch", bufs=3) as prpool,
            tc.tile_pool(name="small", bufs=1) as smpool,
        ):
            # im packed as [p, t*D + d] so j = t*128 + p matches the s tiles.
            # Issued on the ACT HWDGE ring so it overlaps the first s chunk
            # (which streams on the Sync ring).
            im_t = cpool.tile([P, T * D], f32, tag="im")
            nc.scalar.dma_start(
                out=im_t[:].rearrange("p (t d) -> p t d", t=T),
                in_=im_d[:].rearrange("(t p) d -> p t d", p=P),
            )

            # scores^T: partition = column j (within tile t), free = local row i
            scoresT = smpool.tile([P, T * BL], f32, tag="scoresT")

            for ch in range(BL // CHUNK):
                s_t = spool.tile([P, CHUNK * T * D], f32, tag="s")
                nc.sync.dma_start(
                    out=s_t[:].rearrange("p (r t d) -> p r t d", r=CHUNK, t=T),
                    in_=s_d[ch * CHUNK:(ch + 1) * CHUNK].rearrange(
                        "r (t p) d -> p r t d", p=P
                    ),
                )
                for r in range(CHUNK):
                    i = ch * CHUNK + r
                    off = r * T * D
                    prod = prpool.tile([P, T * D], f32, tag="prod")
                    # DVE: one fused product over both column halves.
                    nc.vector.tensor_mul(
                        prod[:], s_t[:, off:off + T * D], im_t[:]
                    )
                    # Free-axis sums: ACT accumulates most halves; DVE
                    # takes every 4th i's t=1 half to balance engine load.
                    nc.scalar.activation(
                        out=prod[:, 0:D], in_=prod[:, 0:D],
                        func=mybir.ActivationFunctionType.Copy,
                        accum_out=scoresT[:, i:i + 1],
                    )
                    if i % 8 == 7:
                        nc.vector.reduce_sum(
                            scoresT[:, BL + i:BL + i + 1], prod[:, D:2 * D],
                            axis=mybir.AxisListType.X,
                        )
                    else:
                        nc.scalar.activation(
                            out=prod[:, D:2 * D], in_=prod[:, D:2 * D],
                            func=mybir.ActivationFunctionType.Copy,
                            accum_out=scoresT[:, BL + i:BL + i + 1],
                        )

            # Masks are only needed for the epilogue — load them late so
            # their DMAs never delay the first s chunk.
            mt_t = cpool.tile([P, T * BL], f32, tag="maskT")
            nc.sync.dma_start(
                out=mt_t[:].rearrange("p (t i) -> p t i", t=T),
                in_=mt_d[:].rearrange("(t p) i -> p t i", p=P),
            )
            nr_t = cpool.tile([BL, B], f32, tag="negrows")
            nc.sync.dma_start(out=nr_t[:], in_=nr_d[:])
            er_t = cpool.tile([BL, B], f32, tag="eyerows")
            nc.sync.dma_start(out=er_t[:], in_=er_d[:])

            # Column maxima over local rows, diagonal masked to -1e30.
            colmax = smpool.tile([P, T], f32, tag="colmax")
            for t in range(T):
                mscr = prpool.tile([P, BL], f32, tag="mscr")
                nc.vector.tensor_add(
                    mscr[:],
                    scoresT[:, t * BL:(t + 1) * BL],
                    mt_t[:, t * BL:(t + 1) * BL],
                )
                nc.vector.reduce_max(
                    colmax[:, t:t + 1], mscr[:], axis=mybir.AxisListType.X
                )

            # Transpose scores^T -> rows [32, 256] via 32x32 stream blocks.
            rows = smpool.tile([BL, B], f32, tag="rows")
            for t in range(T):
                for k in range(P // 32):
                    nc.vector.transpose(
                        out=rows[0:BL, t * P + k * 32:t * P + (k + 1) * 32],
                        in_=scoresT[k * 32:(k + 1) * 32, t * BL:(t + 1) * BL],
                    )

            # rowmax (diag masked), diag, then rowcost = relu(margin + rowmax - diag)
            rowstat = smpool.tile([BL, 4], f32, tag="rowstat")
            rs1 = prpool.tile([BL, B], f32, tag="rscr")
            nc.vector.tensor_add(rs1[:], rows[:], nr_t[:])
            nc.vector.reduce_max(
                rowstat[:, 0:1], rs1[:], axis=mybir.AxisListType.X
            )
            rs2 = prpool.tile([BL, B], f32, tag="rscr")
            nc.vector.tensor_mul(rs2[:], rows[:], er_t[:])
            nc.vector.reduce_sum(
                rowstat[:, 1:2], rs2[:], axis=mybir.AxisListType.X
            )
            nc.vector.tensor_sub(rowstat[:, 2:3], rowstat[:, 0:1], rowstat[:, 1:2])
            nc.vector.tensor_scalar(
                out=rowstat[:, 3:4], in0=rowstat[:, 2:3],
                scalar1=MARGIN, scalar2=0.0, op0=add, op1=amax,
            )

            nc.sync.dma_start(out=rc_d[:], in_=rowstat[:, 3:4])
            nc.sync.dma_start(out=dg_d[:], in_=rowstat[:, 1:2])
            for t in range(T):
                nc.sync.dma_start(
                    out=cm_d[t * P:(t + 1) * P], in_=colmax[:, t:t + 1]
                )

    nc.compile()
    return nc


def _get_nc():
    global _NC
    if _NC is None:
        _NC = _build_nc()
    return _NC


def _make_in_maps(im, s):
    il = np.arange(BL)
    in_maps = []
    for c in range(M):
        mt = np.zeros((B, BL), np.float32)
        nr = np.zeros((BL, B), np.float32)
        er = np.zeros((BL, B), np.float32)
        mt[c * BL + il, il] = NEG
        nr[il, c * BL + il] = NEG
        er[il, c * BL + il] = 1.0
        in_maps.append({
            "im": im,
            "s": s[c * BL:(c + 1) * BL],
            "mask_t_neg": mt,
            "neg_rows": nr,
            "eye_rows": er,
        })
    return in_maps


def _combine(results):
    rowcosts = np.concatenate([results[c]["rowcost"][:, 0] for c in range(M)])
    colmax = np.max(
        np.stack([results[c]["colmax"][:, 0] for c in range(M)]), axis=0
    )
    diag = np.concatenate([results[c]["diagv"][:, 0] for c in range(M)])
    cost_im = np.maximum(np.float32(MARGIN) + colmax - diag, np.float32(0.0))
    loss = rowcosts.sum(dtype=np.float32) + cost_im.sum(dtype=np.float32)
    return np.array(loss, dtype=np.float32)


def _run(im, s, **spmd_kwargs):
    from concourse.bass_utils import run_bass_kernel_spmd

    im = np.ascontiguousarray(np.asarray(im), dtype=np.float32)
    s = np.ascontiguousarray(np.asarray(s), dtype=np.float32)
    nc = _get_nc()
    res = run_bass_kernel_spmd(nc, _make_in_maps(im, s), list(range(M)),
                               **spmd_kwargs)
    return _combine(res.results), res


def kernel(im, s):
    loss, _ = _run(im, s)
    return loss


# revision 12
# speedup vs baseline: 1.1829x; 1.1025x over previous
"""Trainium2 Bass kernel for nn_ContrastiveLoss3DTo2D.

Reference computation (B=256, D=1024, margin=0.2):
    scores[i, j] = dot(im[j], s[i, j])                    # [B, B]
    cost_s  = sum_i relu(margin + max_{j!=i} scores[i,j] - scores[i,i])
    cost_im = sum_j relu(margin + max_{i!=j} scores[i,j] - scores[j,j])
    loss = cost_s + cost_im

Sharding: s (and the score matrix) is sharded along i across 8 cores
(32 rows each); im is replicated. Each core computes its 32x256 score
block via a fused DVE multiply+reduce while streaming its 32 MB shard
from HBM, then reduces on-device to tiny per-core partials:
  rowcost[32]  - relu(margin + rowmax_offdiag - diag) per local row
  colmax[256]  - per-column max over local rows (diagonal masked out)
  diagv[32]    - local diagonal scores
The host combines partials (max over cores for columns, sums) into the
scalar loss. relu/max commute (relu is monotone), so per-core column
maxima compose exactly.
"""

import numpy as np

B = 256
D = 1024
M = 8            # cores
BL = B // M      # 32 local rows per core
P = 128          # SBUF partitions
T = B // P       # 2 column tiles of 128
MARGIN = 0.2
CHUNK = 4        # s rows per DMA (4 MB transfers)
NEG = -1.0e30    # diagonal mask value
NEG_INIT = -3.0e38

_NC = None


def _build_nc():
    import concourse.bacc as bacc
    from concourse import mybir
    from concourse.tile import TileContext

    f32 = mybir.dt.float32
    add = mybir.AluOpType.add
    mult = mybir.AluOpType.mult
    amax = mybir.AluOpType.max

    nc = bacc.Bacc(None, target_bir_lowering=False, debug=False)
    im_d = nc.declare_dram_parameter("im", [B, D], f32, isOutput=False)
    s_d = nc.declare_dram_parameter("s", [BL, B, D], f32, isOutput=False)
    mt_d = nc.declare_dram_parameter("mask_t_neg", [B, BL], f32, isOutput=False)
    nr_d = nc.declare_dram_parameter("neg_rows", [BL, B], f32, isOutput=False)
    er_d = nc.declare_dram_parameter("eye_rows", [BL, B], f32, isOutput=False)
    rc_d = nc.declare_dram_parameter("rowcost", [BL, 1], f32, isOutput=True)
    cm_d = nc.declare_dram_parameter("colmax", [B, 1], f32, isOutput=True)
    dg_d = nc.declare_dram_parameter("diagv", [BL, 1], f32, isOutput=True)

    with TileContext(nc) as tc:
        with (
            tc.tile_pool(name="const", bufs=1) as cpool,
            tc.tile_pool(name="sload", bufs=4) as spool,
            tc.tile_pool(name="scratch", bufs=3) as prpool,
            tc.tile_pool(name="small", bufs=1) as smpool,
        ):
            # im packed as [p, t*D + d] so j = t*128 + p matches the s tiles.
            # First on the Sync ring, ahead of the s stream.
            im_t = cpool.tile([P, T * D], f32, tag="im")
            nc.sync.dma_start(
                out=im_t[:].rearrange("p (t d) -> p t d", t=T),
                in_=im_d[:].rearrange("(t p) d -> p t d", p=P),
            )

            # scores^T: partition = column j (within tile t), free = local row i
            scoresT = smpool.tile([P, T * BL], f32, tag="scoresT")

            # Ramped chunk sizes: small first chunks so the first multiply
            # starts as soon as ~2 MB has landed, then steady 4-row chunks.
            chunk_rows = [1, 1, 2] + [CHUNK] * ((BL - 4) // CHUNK)
            assert sum(chunk_rows) == BL
            row0 = 0
            for nr in chunk_rows:
                s_t = spool.tile([P, CHUNK * T * D], f32, tag="s")
                nc.sync.dma_start(
                    out=s_t[:, 0:nr * T * D].rearrange(
                        "p (r t d) -> p r t d", r=nr, t=T
                    ),
                    in_=s_d[row0:row0 + nr].rearrange(
                        "r (t p) d -> p r t d", p=P
                    ),
                )
                for r in range(nr):
                    i = row0 + r
                    off = r * T * D
                    prod = prpool.tile([P, T * D], f32, tag="prod")
                    # DVE: one fused product over both column halves.
                    nc.vector.tensor_mul(
                        prod[:], s_t[:, off:off + T * D], im_t[:]
                    )
                    # Free-axis sums: ACT accumulates most halves; DVE
                    # takes every 4th i's t=1 half to balance engine load.
                    nc.scalar.activation(
                        out=prod[:, 0:D], in_=prod[:, 0:D],
                        func=mybir.ActivationFunctionType.Copy,
                        accum_out=scoresT[:, i:i + 1],
                    )
                    if i % 4 == 3:
                        nc.vector.reduce_sum(
                            scoresT[:, BL + i:BL + i + 1], prod[:, D:2 * D],
                            axis=mybir.AxisListType.X,
                        )
                    else:
                        nc.scalar.activation(
                            out=prod[:, D:2 * D], in_=prod[:, D:2 * D],
                            func=mybir.ActivationFunctionType.Copy,
                            accum_out=scoresT[:, BL + i:BL + i + 1],
                        )
                row0 += nr

            # Masks are only needed for the epilogue — load them late so
            # their DMAs never delay the first s chunk.
            mt_t = cpool.tile([P, T * BL], f32, tag="maskT")
            nc.sync.dma_start(
                out=mt_t[:].rearrange("p (t i) -> p t i", t=T),
                in_=mt_d[:].rearrange("(t p) i -> p t i", p=P),
            )
            nr_t = cpool.tile([BL, B], f32, tag="negrows")
            nc.sync.dma_start(out=nr_t[:], in_=nr_d[:])
            er_t = cpool.tile([BL, B], f32, tag="eyerows")
            nc.sync.dma_start(out=er_t[:], in_=er_d[:])

            # Column maxima over local rows, diagonal masked to -1e30.
            colmax = smpool.tile([P, T], f32, tag="colmax")
            for t in range(T):
                mscr = prpool.tile([P, BL], f32, tag="mscr")
                nc.vector.tensor_add(
                    mscr[:],
                    scoresT[:, t * BL:(t + 1) * BL],
                    mt_t[:, t * BL:(t + 1) * BL],
                )
                nc.vector.reduce_max(
                    colmax[:, t:t + 1], mscr[:], axis=mybir.AxisListType.X
                )

            # Transpose scores^T -> rows [32, 256] via 32x32 stream blocks.
            rows = smpool.tile([BL, B], f32, tag="rows")
            for t in range(T):
                for k in range(P // 32):
                    nc.vector.transpose(
                        out=rows[0:BL, t * P + k * 32:t * P + (k + 1) * 32],
                        in_=scoresT[k * 32:(k + 1) * 32, t * BL:(t + 1) * BL],
                    )

            # rowmax (diag masked), diag, then rowcost = relu(margin + rowmax - diag)
            rowstat = smpool.tile([BL, 4], f32, tag="rowstat")
            rs1 = prpool.tile([BL, B], f32, tag="rscr")
            nc.vector.tensor_add(rs1[:], rows[:], nr_t[:])
            nc.vector.reduce_max(
                rowstat[:, 0:1], rs1[:], axis=mybir.AxisListType.X
            )
            rs2 = prpool.tile([BL, B], f32, tag="rscr")
            nc.vector.tensor_mul(rs2[:], rows[:], er_t[:])
            nc.vector.reduce_sum(
                rowstat[:, 1:2], rs2[:], axis=mybir.AxisListType.X
            )
            nc.vector.tensor_sub(rowstat[:, 2:3], rowstat[:, 0:1], rowstat[:, 1:2])
            nc.vector.tensor_scalar(
                out=rowstat[:, 3:4], in0=rowstat[:, 2:3],
                scalar1=MARGIN, scalar2=0.0, op0=add, op1=amax,
            )

            nc.sync.dma_start(out=rc_d[:], in_=rowstat[:, 3:4])
            nc.sync.dma_start(out=dg_d[:], in_=rowstat[:, 1:2])
            for t in range(T):
                nc.sync.dma_start(
                    out=cm_d[t * P:(t + 1) * P], in_=colmax[:, t:t + 1]
                )

    nc.compile()
    return nc


def _get_nc():
    global _NC
    if _NC is None:
        _NC = _build_nc()
    return _NC


def _make_in_maps(im, s):
    il = np.arange(BL)
    in_maps = []
    for c in range(M):
        mt = np.zeros((B, BL), np.float32)
        nr = np.zeros((BL, B), np.float32)
        er = np.zeros((BL, B), np.float32)
        mt[c * BL + il, il] = NEG
        nr[il, c * BL + il] = NEG
        er[il, c * BL + il] = 1.0
        in_maps.append({
            "im": im,
            "s": s[c * BL:(c + 1) * BL],
            "mask_t_neg": mt,
            "neg_rows": nr,
            "eye_rows": er,
        })
    return in_maps


def _combine(results):
    rowcosts = np.concatenate([results[c]["rowcost"][:, 0] for c in range(M)])
    colmax = np.max(
        np.stack([results[c]["colmax"][:, 0] for c in range(M)]), axis=0
    )
    diag = np.concatenate([results[c]["diagv"][:, 0] for c in range(M)])
    cost_im = np.maximum(np.float32(MARGIN) + colmax - diag, np.float32(0.0))
    loss = rowcosts.sum(dtype=np.float32) + cost_im.sum(dtype=np.float32)
    return np.array(loss, dtype=np.float32)


def _run(im, s, **spmd_kwargs):
    from concourse.bass_utils import run_bass_kernel_spmd

    im = np.ascontiguousarray(np.asarray(im), dtype=np.float32)
    s = np.ascontiguousarray(np.asarray(s), dtype=np.float32)
    nc = _get_nc()
    res = run_bass_kernel_spmd(nc, _make_in_maps(im, s), list(range(M)),
                               **spmd_kwargs)
    return _combine(res.results), res


def kernel(im, s):
    loss, _ = _run(im, s)
    return loss
